# revision 6
# baseline (speedup 1.0000x reference)
"""Trainium2 Bass kernel for nn_AMLNeuralNetwork3D (dense_mlp).

Strategy: 8-way tensor parallel (column split on output features) for all
three 8192x8192 dense layers; the per-gene local layer shards along the
gene axis (matching the feature split).  After the local layer and after
L1/L2 the per-core feature slices are AllGather'd (concat on partition
axis = gene axis).  L3 slices are returned per-core and assembled on host.

Layout: activations are kept feature-major [features, batch] on chip so a
layer's output layout equals the next layer's input layout (contraction is
over the partition axis on the TensorEngine).  Weights are pre-transposed
on host to [in_features, out_slice] so all DMAs are wide/contiguous.

Compute in bf16 (full-rate on the PE, fp32 PSUM accumulation); measured
L2 rel-err of the full net in bf16 is ~5e-3.
"""

import sys

if "/opt/trn_rl_repo" not in sys.path:
    sys.path.insert(0, "/opt/trn_rl_repo")

import numpy as np
import ml_dtypes

N_CORES = 8
G = 8192          # genes / features
B = 1024          # batch
L = 4             # levels
GS = G // N_CORES # per-core feature slice (1024)
NB = 512          # batch chunk (one PSUM bank at fp32)
NCHUNK = B // NB  # 2
GT = GS // 128    # gene tiles per core slice (8)
KT = G // 128     # contraction tiles (64)

BF16 = ml_dtypes.bfloat16

_compiled = {}


def _build_graph():
    from concourse import bacc, tile
    import concourse.mybir as mybir

    fp32 = mybir.dt.float32
    bf16 = mybir.dt.bfloat16
    Relu = mybir.ActivationFunctionType.Relu
    Identity = mybir.ActivationFunctionType.Identity
    mult = mybir.AluOpType.mult
    add = mybir.AluOpType.add
    bypass = mybir.AluOpType.bypass

    nc = bacc.Bacc(None, target_bir_lowering=False, num_devices=N_CORES)

    # ---- parameters (per-core shards; same graph on all cores) ----
    x_p = nc.declare_dram_parameter("x", [L, GS, B], bf16, isOutput=False)
    # per-feature scalars: cols 0..3 = W_local, 4 = b_local, 5..7 = b1..b3
    scal_p = nc.declare_dram_parameter("scal", [GS, 8], fp32, isOutput=False)
    w_p = [
        nc.declare_dram_parameter(f"w{k}t", [G, GS], bf16, isOutput=False)
        for k in (1, 2, 3)
    ]
    out_p = nc.declare_dram_parameter("out", [GS, B], fp32, isOutput=True)

    rg = [list(range(N_CORES))]

    with tile.TileContext(nc) as tc:
        with (
            tc.tile_pool(name="dram", bufs=1, space="DRAM") as dram,
            tc.tile_pool(name="scal", bufs=GT) as spool,
            tc.tile_pool(name="xin", bufs=12) as xpool,
            tc.tile_pool(name="loc", bufs=10) as lpool,
            tc.tile_pool(name="hin", bufs=28) as hpool,
            tc.tile_pool(name="wblk", bufs=16) as wpool,
            tc.tile_pool(name="hout", bufs=6) as opool,
            tc.tile_pool(name="psum", bufs=8, space="PSUM") as ppool,
        ):
            # bounce buffers for the 3 AllGather transitions x 2 chunks
            slc = [
                [
                    dram.tile([GS, NB], bf16, name=f"slc_{t}_{j}", tag=f"slc_{t}_{j}")
                    for j in range(NCHUNK)
                ]
                for t in range(3)
            ]
            gath = [
                [
                    dram.tile(
                        [G, NB], bf16, name=f"gath_{t}_{j}", tag=f"gath_{t}_{j}",
                        addr_space="Shared",
                    )
                    for j in range(NCHUNK)
                ]
                for t in range(3)
            ]

            # --- PE warmup: keep TensorE busy from t~0 through the prologue
            # (launch barrier + first AllGather). The chip locks its PE clock
            # profile based on early NEFF activity: a NEFF that opens with
            # DVE/ACT/collective work runs all matmuls at 2.0 GHz; one that
            # opens with matmuls runs at 2.4 GHz. These are also free: the PE
            # would otherwise idle until the first gathered tile arrives.
            wu_w = spool.tile([128, 128], bf16, name="wu_w", tag="wu_w")
            nc.sync.dma_start(wu_w[:], w_p[0][0:128, 0:128])
            wu_h = spool.tile([128, NB], bf16, name="wu_h", tag="wu_h")
            nc.sync.dma_start(wu_h[:], w_p[0][0:128, 0:NB])
            wu_ps = ppool.tile([128, NB], fp32, name="wu_ps", tag="ps")
            N_WARMUP = 700
            for i in range(N_WARMUP):
                nc.tensor.matmul(
                    wu_ps[:], wu_w[:], wu_h[:],
                    start=(i == 0), stop=(i == N_WARMUP - 1),
                )
            wu_out = spool.tile([128, NB], bf16, name="wu_out", tag="wu_out")
            nc.scalar.activation(
                wu_out[:], wu_ps[:], mybir.ActivationFunctionType.Copy
            )
            wu_dram = dram.tile([128, NB], bf16, name="wu_dram", tag="wu_dram")
            nc.sync.dma_start(wu_dram[:], wu_out[:])

            # per-feature scalar tiles, persistent
            sc = []
            for gt in range(GT):
                s = spool.tile([128, 8], fp32, name=f"sc{gt}", tag="sc")
                nc.sync.dma_start(s[:], scal_p[gt * 128 : (gt + 1) * 128, :])
                sc.append(s)

            def local_layer(j):
                # levels 0,1 on DVE; levels 2,3 on ACT (ScalarE has its own
                # SBUF ports; GpSimd shares DVE's so it can't help)
                for gt in range(GT):
                    xt = []
                    for l in range(L):
                        t = xpool.tile([128, NB], bf16, name=f"x{j}_{gt}_{l}", tag="x")
                        nc.sync.dma_start(
                            t[:],
                            x_p[l, gt * 128 : (gt + 1) * 128, j * NB : (j + 1) * NB],
                        )
                        xt.append(t)
                    t2 = lpool.tile([128, NB], bf16, name=f"t2_{j}_{gt}", tag="acc")
                    nc.scalar.activation(
                        t2[:], xt[2][:], Identity, scale=sc[gt][:, 2:3]
                    )
                    t3 = lpool.tile([128, NB], bf16, name=f"t3_{j}_{gt}", tag="acc")
                    nc.scalar.activation(
                        t3[:], xt[3][:], Identity, scale=sc[gt][:, 3:4]
                    )
                    m = lpool.tile([128, NB], bf16, name=f"m0_{j}_{gt}", tag="acc")
                    nc.vector.tensor_scalar(m[:], xt[0][:], sc[gt][:, 0:1], None, mult)
                    m1 = lpool.tile([128, NB], bf16, name=f"m1_{j}_{gt}", tag="acc")
                    nc.vector.scalar_tensor_tensor(
                        m1[:], xt[1][:], sc[gt][:, 1:2], m[:], mult, add
                    )
                    m2 = lpool.tile([128, NB], bf16, name=f"m2_{j}_{gt}", tag="acc")
                    nc.vector.tensor_tensor(m2[:], m1[:], t2[:], add)
                    m3 = lpool.tile([128, NB], bf16, name=f"m3_{j}_{gt}", tag="acc")
                    nc.vector.tensor_tensor(m3[:], m2[:], t3[:], add)
                    h0 = opool.tile([128, NB], bf16, name=f"h0_{j}_{gt}", tag="hout")
                    nc.scalar.activation(h0[:], m3[:], Relu, bias=sc[gt][:, 4:5])
                    nc.sync.dma_start(slc[0][j][gt * 128 : (gt + 1) * 128, :], h0[:])

            def dense_layer(k, j):
                # k in {1,2,3}; input from gath[k-1][j]; output slice ->
                # slc[k][j] (k<3) or out_p (k==3)
                src = gath[k - 1][j]
                wt = w_p[k - 1]
                ps = [
                    ppool.tile([128, NB], fp32, name=f"ps{k}_{j}_{o}", tag="ps")
                    for o in range(GT)
                ]
                for g in range(KT):
                    ht = hpool.tile([128, NB], bf16, name=f"h{k}_{j}_{g}", tag="hin")
                    nc.sync.dma_start(ht[:], src[g * 128 : (g + 1) * 128, :])
                    wb = wpool.tile([128, GS], bf16, name=f"w{k}_{j}_{g}", tag="wblk")
                    nc.sync.dma_start(wb[:], wt[g * 128 : (g + 1) * 128, :])
                    for o in range(GT):
                        nc.tensor.matmul(
                            ps[o][:],
                            wb[:, o * 128 : (o + 1) * 128],
                            ht[:],
                            start=(g == 0),
                            stop=(g == KT - 1),
                        )
                for o in range(GT):
                    if k < 3:
                        ot = opool.tile(
                            [128, NB], bf16, name=f"o{k}_{j}_{o}", tag="hout"
                        )
                        nc.scalar.activation(
                            ot[:], ps[o][:], Relu, bias=sc[o][:, 4 + k : 5 + k]
                        )
                        nc.sync.dma_start(
                            slc[k][j][o * 128 : (o + 1) * 128, :], ot[:]
                        )
                    else:
                        ot = opool.tile(
                            [128, NB], fp32, name=f"o{k}_{j}_{o}", tag="outp"
                        )
                        nc.scalar.activation(
                            ot[:], ps[o][:], Relu, bias=sc[o][:, 7:8]
                        )
                        nc.sync.dma_start(
                            out_p[o * 128 : (o + 1) * 128, j * NB : (j + 1) * NB],
                            ot[:],
                        )

            def allgather(t, j):
                nc.gpsimd.collective_compute(
                    "AllGather",
                    bypass,
                    replica_groups=rg,
                    ins=[slc[t][j][:].opt()],
                    outs=[gath[t][j][:].opt()],
                )

            # emission order = desired overlap order
            local_layer(0)
            allgather(0, 0)
            local_layer(1)
            allgather(0, 1)
            for k in (1, 2, 3):
                for j in range(NCHUNK):
                    dense_layer(k, j)
                    if k < 3:
                        allgather(k, j)

    nc.compile()
    return nc


def _get_nc():
    if "nc" not in _compiled:
        _compiled["nc"] = _build_graph()
    return _compiled["nc"]


def kernel(x, W_local, b_local, W1, b1, W2, b2, W3, b3):
    from concourse.bass_utils import run_bass_kernel_spmd

    nc = _get_nc()

    x = np.asarray(x)
    in_maps = []
    for r in range(N_CORES):
        sl = slice(r * GS, (r + 1) * GS)
        x_r = x[:, :, sl].transpose(0, 2, 1).astype(BF16)
        scal_r = np.concatenate(
            [
                np.asarray(W_local)[sl, :],
                np.asarray(b_local)[sl, None],
                np.asarray(b1)[sl, None],
                np.asarray(b2)[sl, None],
                np.asarray(b3)[sl, None],
            ],
            axis=1,
        ).astype(np.float32)
        in_maps.append(
            {
                "x": x_r,
                "scal": np.ascontiguousarray(scal_r),
                "w1t": np.asarray(W1)[sl, :].T.astype(BF16),
                "w2t": np.asarray(W2)[sl, :].T.astype(BF16),
                "w3t": np.asarray(W3)[sl, :].T.astype(BF16),
            }
        )

    res = run_bass_kernel_spmd(nc, in_maps, core_ids=list(range(N_CORES)))

    out = np.empty((B, G), np.float32)
    for r in range(N_CORES):
        out[:, r * GS : (r + 1) * GS] = res.results[r]["out"].T
    return out
